# revision 3
# baseline (speedup 1.0000x reference)
"""Trainium2 Bass kernel for nn_Decoder (2-layer RNN decoder).

Reference computation (per layer, scanned over T):
    c = concat([x_t, h], 1); h' = tanh(c @ Wh + bh); o = tanh(c @ Wo + bo)
Layer 0 h0 = encoder_output, layer 1 h0 = 0, output = layer-1 o.

Strategy (per core, batch shard of 8):
  - the two layer recurrences are OVERLAPPED in one superstep loop with a
    lag of LAG steps: superstep s runs layer-0 step s and layer-1 step
    s-LAG. Each recurrence's serial chain (matmul drain -> sem -> tanh ->
    sem) hides under the other's PE weight loads.
  - per step per layer: the DVE pre-writes P_t = x_t @ Whx + bh (bf16)
    into the step's PSUM bank (has_written bits armed once by a warmup
    matmul per bank), then 16 bf16 Whh-tile matmuls accumulate on top
    (weight-load bound at ~27-32ns/tile with FWL); ScalarE tanh writes
    the transposed hidden store.
  - all fill GEMMs (P precompute, o0 outputs, final out) run in bf16 and
    are interleaved into the loop via a deadline-sorted FIFO: emission
    order forms Tile dependencies, so every P/x writer item is emitted
    (mandatorily drained) before the rec step that reads it; read-only
    final-out blocks may skip ahead into idle supersteps.
  - o0/P1 are produced in 256-column blocks (32 steps) so layer 1 can
    start after LAG=64 steps of layer 0; P epilogues run on the DVE.

Sharding: data-parallel over batch (B=64 -> 8 cores x 8), weights replicated.
"""
import sys

if "/opt/trn_rl_repo" not in sys.path:
    sys.path.insert(0, "/opt/trn_rl_repo")

import numpy as np
from contextlib import ExitStack

import concourse.bacc as bacc
import concourse.mybir as mybir
import concourse.tile as tile
from concourse.bass_utils import run_bass_kernel_spmd
from concourse.masks import make_identity
from concourse.tile_rust import add_dep_helper

F32 = mybir.dt.float32
BF16 = mybir.dt.bfloat16
Tanh = mybir.ActivationFunctionType.Tanh
ADD = mybir.AluOpType.add

B_LOC = 8          # batch per core
D = 512            # input feature dim
H = 512            # hidden dim
KC = 4             # 128-chunks in D or H
N_CORES = 8
LAG = 56           # layer-1 recurrence lag (supersteps)


def build_kernel(T=256):
    """Build the per-core Bass program (fully unrolled, Tile-scheduled)."""
    assert T % 64 == 0
    TB = T * B_LOC                 # time-major column count (t*8+b)
    NB = TB // 512                 # 512-wide blocks (P0 GEMM)
    NB2 = TB // 256                # 256-wide blocks (o0 / P1 GEMMs)
    MT = TB // 128                 # 128-row output chunks (final out)
    NG = MT // 4                   # 512-col x-load groups
    HS = TB + B_LOC                # hidden store column count (h_{-1}..h_{T-1})
    S_TOT = T + LAG                # total supersteps

    nc = bacc.Bacc(None)
    x_d = nc.dram_tensor("x", [B_LOC, T, D], F32, kind="ExternalInput")
    enc_d = nc.dram_tensor("encoder_output", [B_LOC, H], F32, kind="ExternalInput")
    wh0_d = nc.dram_tensor("Wh0", [D + H, H], F32, kind="ExternalInput")
    bh0_d = nc.dram_tensor("bh0", [H], F32, kind="ExternalInput")
    wo0_d = nc.dram_tensor("Wo0", [D + H, D], F32, kind="ExternalInput")
    bo0_d = nc.dram_tensor("bo0", [D], F32, kind="ExternalInput")
    wh1_d = nc.dram_tensor("Wh1", [D + H, H], F32, kind="ExternalInput")
    bh1_d = nc.dram_tensor("bh1", [H], F32, kind="ExternalInput")
    wo1_d = nc.dram_tensor("Wo1", [D + H, D], F32, kind="ExternalInput")
    bo1_d = nc.dram_tensor("bo1", [D], F32, kind="ExternalInput")
    out_d = nc.dram_tensor("out", [B_LOC, T, D], F32, kind="ExternalOutput")

    with tile.TileContext(nc) as tc, ExitStack() as ctx:
        sb = ctx.enter_context(tc.tile_pool(name="sb", bufs=1))
        stg = ctx.enter_context(tc.tile_pool(name="stg", bufs=2))
        xstg = ctx.enter_context(tc.tile_pool(name="xstg", bufs=2))
        ps_g = ctx.enter_context(tc.tile_pool(name="ps_g", bufs=2, space="PSUM"))
        ps_t = ctx.enter_context(tc.tile_pool(name="ps_t", bufs=2, space="PSUM"))
        ps_z0 = ctx.enter_context(tc.tile_pool(name="ps_z0", bufs=2, space="PSUM"))
        ps_z1 = ctx.enter_context(tc.tile_pool(name="ps_z1", bufs=2, space="PSUM"))

        # ---------- constants ----------
        ident = sb.tile([128, 128], F32, tag="ident", name="ident")
        make_identity(nc, ident[:])
        ident_b = sb.tile([128, 128], BF16, tag="ident_b", name="ident_b")
        nc.vector.tensor_copy(ident_b[:], ident[:])

        # ---------- biases ----------
        def load_bias_cols(dram, tag):
            t_ = sb.tile([128, KC], F32, tag=tag, name=tag)
            nc.sync.dma_start(t_[:], dram[:].rearrange("(c p) -> p c", p=128))
            return t_

        bh0 = load_bias_cols(bh0_d, "bh0")
        bo0 = load_bias_cols(bo0_d, "bo0")
        bh1 = load_bias_cols(bh1_d, "bh1")
        bo1f = sb.tile([1, 512], F32, tag="bo1f", name="bo1f")
        nc.sync.dma_start(bo1f[:], bo1_d[:].rearrange("(o n) -> o n", o=1))
        bo1b = sb.tile([1, 512], BF16, tag="bo1b", name="bo1b")
        nc.vector.tensor_copy(bo1b[:], bo1f[:])
        ones_f = sb.tile([1, 128], F32, tag="ones_f", name="ones_f")
        nc.vector.memset(ones_f[:], 1.0)
        ones_b = sb.tile([1, 128], BF16, tag="ones_b", name="ones_b")
        nc.vector.tensor_copy(ones_b[:], ones_f[:])

        # ---------- weights (all bf16) ----------
        # layout per weight half: [128, k*512 + m*128 + col] (k = K-chunk of
        # the contraction dim, m = 128-chunk of output features)
        def load_half(dram, row0, tag, q=None):
            w = sb.tile([128, KC * 512], BF16, tag=tag, name=tag)
            s = stg.tile([128, KC * 512], F32, tag="stag", name="stag")
            eng = q if q is not None else nc.sync
            for k in range(KC):
                eng.dma_start(
                    s[:, k * 512:(k + 1) * 512],
                    dram[row0 + k * 128: row0 + (k + 1) * 128, :])
            nc.vector.tensor_copy(w[:], s[:])
            return w

        # ---------- hidden-state stores [128, k*HS + col], col t = h_{t-1} ----------
        h0T = sb.tile([128, KC * HS], BF16, tag="h0T", name="h0T")
        h1T = sb.tile([128, KC * HS], BF16, tag="h1T", name="h1T")
        encs = stg.tile([B_LOC, H], F32, tag="encs", name="encs")
        nc.sync.dma_start(encs[:], enc_d[:])
        for k in range(KC):
            pt = ps_t.tile([128, B_LOC], F32, tag="pt", name="pt")
            nc.tensor.transpose(pt[:], encs[:, k * 128:(k + 1) * 128],
                                ident[0:B_LOC, 0:B_LOC])
            nc.vector.tensor_copy(h0T[:, k * HS: k * HS + B_LOC], pt[:])
        for k in range(KC):
            nc.vector.memset(h1T[:, k * HS: k * HS + B_LOC], 0.0)

        # ---------- x load + transpose to xT[k] = [128, TB] bf16 ----------
        xT = [sb.tile([128, TB], BF16, tag=f"xT{k}", name=f"xT{k}")
              for k in range(KC)]

        xg_box = {}

        def load_x_group(g, q=None):
            """DMA 4 j-blocks (512 TB-cols) of x into one staging buffer."""
            xs = xstg.tile([128, 2048], F32, tag="xg", name="xg")
            eng = q if q is not None else nc.sync
            for j in range(4):
                t0 = g * 64 + j * 16
                eng.dma_start(
                    xs[:, j * 512:(j + 1) * 512],
                    x_d[:, t0:t0 + 16, :].rearrange("b t d -> t b d"))
            xg_box[g] = xs

        def transpose_x_chunk(g, j):
            """Transpose 128-col chunk j (0..3) of group g into xT."""
            xs = xg_box[g]
            jj = g * 4 + j
            for k in range(KC):
                pt = ps_t.tile([128, 128], F32, tag="pt", name="pt")
                nc.tensor.transpose(
                    pt[:], xs[:, j * 512 + k * 128: j * 512 + (k + 1) * 128],
                    ident[:])
                nc.vector.tensor_copy(
                    xT[k][:, jj * 128:(jj + 1) * 128], pt[:])

        # DMA order on the sync queue: x-g0 first (gates the transposes),
        # then wx0 (gates P0 n=0), then whh0 (gates rec0 step 0)
        load_x_group(0, q=nc.scalar)
        wx0 = load_half(wh0_d, 0, "wx0")
        whh0 = load_half(wh0_d, D, "whh0")
        for j in range(4):
            transpose_x_chunk(0, j)

        # ---------- P stores (single bf16): P = X @ Whx + bh ----------
        P0 = sb.tile([128, T * 32], BF16, tag="P0", name="P0")
        P1 = sb.tile([128, T * 32], BF16, tag="P1", name="P1")

        def p_view(P):
            return P[:].rearrange("p (t m b) -> p t m b", m=KC, b=B_LOC)

        def emit_p_block(P, w, src, bias, m, n, wcols):
            """One (m, n) block of a P GEMM; returns list of emit thunks."""
            thunks = []
            pg_box = []
            ts = wcols // B_LOC

            def mk_mm(k):
                def f(dep=None):
                    if k == 0:
                        pg_box.append(ps_g.tile([128, wcols], F32, tag="pg",
                                                name="pg"))
                    mm = nc.tensor.matmul(
                        pg_box[0][:],
                        w[:, k * 512 + m * 128: k * 512 + (m + 1) * 128],
                        src[k][:, n * wcols:(n + 1) * wcols],
                        start=(k == 0), stop=(k == KC - 1),
                        skip_group_check=True)
                    if dep is not None:
                        add_dep_helper(mm.ins, dep, sync=False, reason="spread")
                return f

            for k in range(KC):
                thunks.append(mk_mm(k))

            def epi(dep=None):
                pg = pg_box[0]
                nc.vector.tensor_scalar_add(
                    p_view(P)[:, n * ts:(n + 1) * ts, m, :],
                    pg[:].rearrange("p (t b) -> p t b", b=B_LOC),
                    bias[:, m: m + 1])
            thunks.append(epi)
            return thunks

        # ---------- o GEMM: tanh(X@Wox + Hprev@Woh + bo) -> bf16 [feat, TB] ----------
        def emit_o_block(dst, wx, wh, hT, bias, m, n, wcols):
            thunks = []
            pg_box = []

            def mk_x(k):
                def f(dep=None):
                    if k == 0:
                        pg_box.append(ps_g.tile([128, wcols], F32, tag="pg",
                                                name="pg"))
                    mm = nc.tensor.matmul(
                        pg_box[0][:],
                        wx[:, k * 512 + m * 128: k * 512 + (m + 1) * 128],
                        xT[k][:, n * wcols:(n + 1) * wcols],
                        start=(k == 0), stop=False, skip_group_check=True)
                    if dep is not None:
                        add_dep_helper(mm.ins, dep, sync=False, reason="spread")
                return f

            def mk_h(k):
                def f(dep=None):
                    mm = nc.tensor.matmul(
                        pg_box[0][:],
                        wh[:, k * 512 + m * 128: k * 512 + (m + 1) * 128],
                        hT[:, k * HS + n * wcols: k * HS + (n + 1) * wcols],
                        start=False, stop=(k == KC - 1), skip_group_check=True)
                    if dep is not None:
                        add_dep_helper(mm.ins, dep, sync=False, reason="spread")
                return f

            for k in range(KC):
                thunks.append(mk_x(k))
            for k in range(KC):
                thunks.append(mk_h(k))

            def epi(dep=None):
                nc.scalar.activation(dst[m][:, n * wcols:(n + 1) * wcols],
                                     pg_box[0][:], Tanh, bias=bias[:, m: m + 1])
            thunks.append(epi)
            return thunks

        out0T = [sb.tile([128, TB], BF16, tag=f"o0T{m}", name=f"o0T{m}")
                 for m in range(KC)]

        # ---------- final output block ([TB, feat] row-major) ----------
        def emit_out_block(mt):
            thunks = []
            po_box = []

            def bias_mm(dep=None):
                po_box.append(ps_g.tile([128, 512], F32, tag="pg", name="pg"))
                mm = nc.tensor.matmul(po_box[0][:], ones_b[:], bo1b[:],
                                 start=True, stop=False, skip_group_check=True)
                if dep is not None:
                    add_dep_helper(mm.ins, dep, sync=False, reason="spread")
            thunks.append(bias_mm)

            def mk_x(k):
                def f(dep=None):
                    mm = nc.tensor.matmul(
                        po_box[0][:], out0T[k][:, mt * 128:(mt + 1) * 128],
                        wo1x[:, k * 512:(k + 1) * 512],
                        start=False, stop=False, skip_group_check=True)
                    if dep is not None:
                        add_dep_helper(mm.ins, dep, sync=False, reason="spread")
                return f

            def mk_h(k):
                def f(dep=None):
                    mm = nc.tensor.matmul(
                        po_box[0][:],
                        h1T[:, k * HS + mt * 128: k * HS + (mt + 1) * 128],
                        woh1[:, k * 512:(k + 1) * 512],
                        start=False, stop=(k == KC - 1), skip_group_check=True)
                    if dep is not None:
                        add_dep_helper(mm.ins, dep, sync=False, reason="spread")
                return f

            for k in range(KC):
                thunks.append(mk_x(k))
            for k in range(KC):
                thunks.append(mk_h(k))

            def epi(dep=None):
                orow = stg.tile([128, 512], F32, tag="orow", name="orow")
                nc.scalar.activation(orow[:], po_box[0][:], Tanh)
                nc.sync.dma_start(
                    out_d[:, mt * 16:(mt + 1) * 16, :].rearrange("b t d -> t b d"),
                    orow[:])
            thunks.append(epi)
            return thunks

        # ---------- P0 n=0 up-front (needed from step 0) ----------
        for m in range(KC):
            for fn in emit_p_block(P0, wx0, xT, bh0, m, 0, 512):
                fn()

        # ---------- remaining DMAs (queue behind the prologue x/weights) ----------
        wox0 = load_half(wo0_d, 0, "wox0")
        woh0 = load_half(wo0_d, D, "woh0")
        if NG > 1:
            load_x_group(1)
        wx1 = load_half(wh1_d, 0, "wx1")
        whh1 = load_half(wh1_d, D, "whh1")
        wo1x = load_half(wo1_d, 0, "wo1x")
        woh1 = load_half(wo1_d, D, "woh1")

        # ---------- recurrence step ----------
        # z-bank warmup: one start=True matmul per z buffer slot sets the
        # bank's has_written bits. After that, every step's DVE prewrite
        # overwrites the memory with P_t (bits stay set), and the whh
        # matmuls (all flags=0) accumulate on top — no identity matmuls on
        # the PE critical path.
        for zpool in (ps_z0, ps_z1):
            for _ in range(2):
                zw = zpool.tile([128, 32], F32, tag="z", name="z")
                nc.tensor.matmul(
                    zw[:], ident_b[:], ident_b[:, 0:32],
                    start=True, stop=True, skip_group_check=True)

        def rec_step(zpool, P, hTa, whh, t):
            hview = hTa[:].rearrange("p (c s) -> p c s", c=KC)
            z = zpool.tile([128, 32], F32, tag="z", name="z")
            nc.vector.tensor_copy(
                z[:].rearrange("p (m b) -> p m b", b=B_LOC),
                p_view(P)[:, t, :, :])
            for k in range(KC):
                for m in range(KC):
                    nc.tensor.matmul(
                        z[:, m * 8:(m + 1) * 8],
                        whh[:, k * 512 + m * 128: k * 512 + (m + 1) * 128],
                        hTa[:, k * HS + t * 8: k * HS + (t + 1) * 8],
                        start=False, stop=(k == KC - 1 and m == KC - 1),
                        skip_group_check=True)
            act = nc.scalar.activation(
                hview[:, :, (t + 1) * 8:(t + 2) * 8],
                z[:].rearrange("p (c b) -> p c b", b=B_LOC),
                Tanh)
            return act

        # ---------- fill schedule ----------
        # Emission ORDER is what forms Tile dependencies: every fill thunk
        # that WRITES data a recurrence step reads (P epilogues, x
        # transposes) must be EMITTED before that step's instructions.
        # Items carry (earliest, deadline): `earliest` = first superstep at
        # which emission is dependency-correct (all writers this item reads
        # are already emitted); `deadline` = superstep of the first consumer
        # (mandatory drain happens just before it). FIFO is deadline-sorted;
        # opportunistic emission only from the head item (never skips).
        INF = 10 ** 9
        items = []   # (deadline, seq, earliest, [thunks])

        def add_item(earliest, deadline, thunks):
            items.append([deadline, len(items), earliest, list(thunks)])

        # x groups g>=1: DMA then transposes (consumed by P0 block g and
        # o0 blocks 2g/2g+1; all deadline-protected via P0's deadline 64g)
        for g in range(1, NG):
            def mk_dma(gg):
                def f(dep=None):
                    load_x_group(gg)
                return f
            def mk_tr(gg, jj):
                def f(dep=None):
                    transpose_x_chunk(gg, jj)
                return f
            thl = [] if g == 1 else [mk_dma(g)]
            thl += [mk_tr(g, j) for j in range(4)]
            add_item(20 * (g - 1) + 10, 64 * g - 8, thl)

        # deferred P0 blocks (cols n*64.. needed from rec0 step n*64)
        for n in range(1, NB):
            thl = []
            for m in range(KC):
                thl += emit_p_block(P0, wx0, xT, bh0, m, n, 512)
            add_item(20 * (n - 1) + 26, 64 * n, thl)

        # o0 + P1 blocks (n2-th reads h0 through step (n2+1)*32 - 1, so
        # earliest emission is ss 32*(n2+1); P1 block n2 is read by rec1
        # step 32*n2 at ss LAG + 32*n2)
        for n2 in range(NB2):
            thl = []
            for m in range(KC):
                thl += emit_o_block(out0T, wox0, woh0, h0T, bo0, m, n2, 256)
            for m in range(KC):
                thl += emit_p_block(P1, wx1, out0T, bh1, m, n2, 256)
            add_item(32 * (n2 + 1), LAG + 32 * n2, thl)

        # final out blocks (read-only consumers of h1T/out0T; mt-th reads
        # h1 through step (mt+1)*16 - 1, emitted at ss LAG+16*(mt+1)-1)
        for mt in range(MT):
            add_item(LAG + 16 * (mt + 1), INF, emit_out_block(mt))

        items.sort(key=lambda it: (it[0], it[1]))

        # ---------- merged superstep loop ----------
        # Opportunistic emission pulls from the FIFO head; additionally ONE
        # read-only OUT item (deadline INF, writers all emitted before its
        # earliest) may run as a skip-ahead stream when the head is not yet
        # eligible. At most two fill blocks are ever open at once, matching
        # ps_g's two buffers.
        CAP = 3
        skip_it = None
        for s in range(S_TOT):
            pend = []
            # mandatory: drain everything whose consumer runs this superstep
            while items and items[0][0] <= s:
                pend += items.pop(0)[3]
            act0 = act1 = None
            if s < T:
                act0 = rec_step(ps_z0, P0, h0T, whh0, s)
            cap = CAP if s < T else 6
            budget = max(0, cap - len(pend))
            while budget > 0:
                if items and items[0][2] <= s:
                    head = items[0][3]
                    pend.append(head.pop(0))
                    budget -= 1
                    if not head:
                        items.pop(0)
                    continue
                if skip_it is None:
                    for it in items:
                        if it[0] >= INF and it[2] <= s:
                            skip_it = it
                            break
                if skip_it is not None:
                    pend.append(skip_it[3].pop(0))
                    budget -= 1
                    if not skip_it[3]:
                        items.remove(skip_it)
                        skip_it = None
                    continue
                break
            dep0 = act0.ins if act0 is not None else None
            for th in pend:
                th(dep0)
            if LAG <= s < T + LAG:
                act1 = rec_step(ps_z1, P1, h1T, whh1, s - LAG)
            if act0 is None and pend:
                # fills on tail-only supersteps hang off act1 instead
                pass

        for it in items:
            for th in it[3]:
                th()

    nc.compile()
    return nc


_NC_CACHE = {}


def _get_nc(T=256):
    if T not in _NC_CACHE:
        _NC_CACHE[T] = build_kernel(T)
    return _NC_CACHE[T]


def kernel(**inputs):
    x = np.ascontiguousarray(inputs["x"], dtype=np.float32)
    enc = np.ascontiguousarray(inputs["encoder_output"], dtype=np.float32)
    B, T, _ = x.shape
    nc = _get_nc(T)
    shared = {
        "Wh0": np.ascontiguousarray(inputs["Wh0"], np.float32),
        "bh0": np.ascontiguousarray(inputs["bh0"], np.float32),
        "Wo0": np.ascontiguousarray(inputs["Wo0"], np.float32),
        "bo0": np.ascontiguousarray(inputs["bo0"], np.float32),
        "Wh1": np.ascontiguousarray(inputs["Wh1"], np.float32),
        "bh1": np.ascontiguousarray(inputs["bh1"], np.float32),
        "Wo1": np.ascontiguousarray(inputs["Wo1"], np.float32),
        "bo1": np.ascontiguousarray(inputs["bo1"], np.float32),
    }
    in_maps = []
    for c in range(N_CORES):
        in_maps.append({
            "x": x[c * B_LOC:(c + 1) * B_LOC],
            "encoder_output": enc[c * B_LOC:(c + 1) * B_LOC],
            **shared,
        })
    res = run_bass_kernel_spmd(nc, in_maps, core_ids=list(range(N_CORES)))
    out = np.concatenate([res.results[c]["out"] for c in range(N_CORES)], axis=0)
    return out.astype(np.float32)


# revision 4
# speedup vs baseline: 1.0133x; 1.0133x over previous
"""Trainium2 Bass kernel for nn_Decoder (2-layer RNN decoder).

Reference computation (per layer, scanned over T):
    c = concat([x_t, h], 1); h' = tanh(c @ Wh + bh); o = tanh(c @ Wo + bo)
Layer 0 h0 = encoder_output, layer 1 h0 = 0, output = layer-1 o.

Strategy (per core, batch shard of 8):
  - the two layer recurrences are OVERLAPPED in one superstep loop with a
    lag of LAG steps: superstep s runs layer-0 step s and layer-1 step
    s-LAG. Each recurrence's serial chain (matmul drain -> sem -> tanh ->
    sem) hides under the other's PE weight loads.
  - per step per layer: the DVE pre-writes P_t = x_t @ Whx + bh (bf16)
    into the step's PSUM bank (has_written bits armed once by a warmup
    matmul per bank), then 16 bf16 Whh-tile matmuls accumulate on top
    (weight-load bound at ~27-32ns/tile with FWL); ScalarE tanh writes
    the transposed hidden store.
  - all fill GEMMs (P precompute, o0 outputs, final out) run in bf16 and
    are interleaved into the loop via a deadline-sorted FIFO: emission
    order forms Tile dependencies, so every P/x writer item is emitted
    (mandatorily drained) before the rec step that reads it; read-only
    final-out blocks may skip ahead into idle supersteps.
  - o0/P1 are produced in 256-column blocks (32 steps) so layer 1 can
    start after LAG=64 steps of layer 0; P epilogues run on the DVE.

Sharding: data-parallel over batch (B=64 -> 8 cores x 8), weights replicated.
"""
import sys

if "/opt/trn_rl_repo" not in sys.path:
    sys.path.insert(0, "/opt/trn_rl_repo")

import numpy as np
from contextlib import ExitStack

import concourse.bacc as bacc
import concourse.mybir as mybir
import concourse.tile as tile
from concourse.bass_utils import run_bass_kernel_spmd
from concourse.masks import make_identity
from concourse.tile_rust import add_dep_helper

F32 = mybir.dt.float32
BF16 = mybir.dt.bfloat16
Tanh = mybir.ActivationFunctionType.Tanh
ADD = mybir.AluOpType.add

B_LOC = 8          # batch per core
D = 512            # input feature dim
H = 512            # hidden dim
KC = 4             # 128-chunks in D or H
N_CORES = 8
LAG = 64           # layer-1 recurrence lag (supersteps)


def build_kernel(T=256):
    """Build the per-core Bass program (fully unrolled, Tile-scheduled)."""
    assert T % 64 == 0
    TB = T * B_LOC                 # time-major column count (t*8+b)
    NB = TB // 512                 # 512-wide blocks (P0 GEMM)
    NB2 = TB // 256                # 256-wide blocks (o0 / P1 GEMMs)
    MT = TB // 128                 # 128-row output chunks (final out)
    NG = MT // 4                   # 512-col x-load groups
    HS = TB + B_LOC                # hidden store column count (h_{-1}..h_{T-1})
    S_TOT = T + LAG                # total supersteps

    nc = bacc.Bacc(None)
    x_d = nc.dram_tensor("x", [B_LOC, T, D], F32, kind="ExternalInput")
    enc_d = nc.dram_tensor("encoder_output", [B_LOC, H], F32, kind="ExternalInput")
    wh0_d = nc.dram_tensor("Wh0", [D + H, H], F32, kind="ExternalInput")
    bh0_d = nc.dram_tensor("bh0", [H], F32, kind="ExternalInput")
    wo0_d = nc.dram_tensor("Wo0", [D + H, D], F32, kind="ExternalInput")
    bo0_d = nc.dram_tensor("bo0", [D], F32, kind="ExternalInput")
    wh1_d = nc.dram_tensor("Wh1", [D + H, H], F32, kind="ExternalInput")
    bh1_d = nc.dram_tensor("bh1", [H], F32, kind="ExternalInput")
    wo1_d = nc.dram_tensor("Wo1", [D + H, D], F32, kind="ExternalInput")
    bo1_d = nc.dram_tensor("bo1", [D], F32, kind="ExternalInput")
    out_d = nc.dram_tensor("out", [B_LOC, T, D], F32, kind="ExternalOutput")

    with tile.TileContext(nc) as tc, ExitStack() as ctx:
        sb = ctx.enter_context(tc.tile_pool(name="sb", bufs=1))
        stg = ctx.enter_context(tc.tile_pool(name="stg", bufs=2))
        xstg = ctx.enter_context(tc.tile_pool(name="xstg", bufs=2))
        ps_g = ctx.enter_context(tc.tile_pool(name="ps_g", bufs=2, space="PSUM"))
        ps_t = ctx.enter_context(tc.tile_pool(name="ps_t", bufs=2, space="PSUM"))
        ps_z0 = ctx.enter_context(tc.tile_pool(name="ps_z0", bufs=2, space="PSUM"))
        ps_z1 = ctx.enter_context(tc.tile_pool(name="ps_z1", bufs=2, space="PSUM"))

        # ---------- constants ----------
        ident = sb.tile([128, 128], F32, tag="ident", name="ident")
        make_identity(nc, ident[:])
        ident_b = sb.tile([128, 128], BF16, tag="ident_b", name="ident_b")
        nc.vector.tensor_copy(ident_b[:], ident[:])

        # ---------- biases ----------
        def load_bias_cols(dram, tag):
            t_ = sb.tile([128, KC], F32, tag=tag, name=tag)
            nc.sync.dma_start(t_[:], dram[:].rearrange("(c p) -> p c", p=128))
            return t_

        bh0 = load_bias_cols(bh0_d, "bh0")
        bo0 = load_bias_cols(bo0_d, "bo0")
        bh1 = load_bias_cols(bh1_d, "bh1")
        bo1f = sb.tile([1, 512], F32, tag="bo1f", name="bo1f")
        nc.sync.dma_start(bo1f[:], bo1_d[:].rearrange("(o n) -> o n", o=1))
        bo1b = sb.tile([1, 512], BF16, tag="bo1b", name="bo1b")
        nc.vector.tensor_copy(bo1b[:], bo1f[:])
        ones_f = sb.tile([1, 128], F32, tag="ones_f", name="ones_f")
        nc.vector.memset(ones_f[:], 1.0)
        ones_b = sb.tile([1, 128], BF16, tag="ones_b", name="ones_b")
        nc.vector.tensor_copy(ones_b[:], ones_f[:])

        # ---------- weights (all bf16) ----------
        # layout per weight half: [128, k*512 + m*128 + col] (k = K-chunk of
        # the contraction dim, m = 128-chunk of output features)
        def load_half(dram, row0, tag, q=None):
            w = sb.tile([128, KC * 512], BF16, tag=tag, name=tag)
            s = stg.tile([128, KC * 512], F32, tag="stag", name="stag")
            eng = q if q is not None else nc.sync
            for k in range(KC):
                eng.dma_start(
                    s[:, k * 512:(k + 1) * 512],
                    dram[row0 + k * 128: row0 + (k + 1) * 128, :])
            nc.vector.tensor_copy(w[:], s[:])
            return w

        # ---------- hidden-state stores [128, k*HS + col], col t = h_{t-1} ----------
        h0T = sb.tile([128, KC * HS], BF16, tag="h0T", name="h0T")
        h1T = sb.tile([128, KC * HS], BF16, tag="h1T", name="h1T")
        encs = stg.tile([B_LOC, H], F32, tag="encs", name="encs")
        nc.sync.dma_start(encs[:], enc_d[:])
        for k in range(KC):
            pt = ps_t.tile([128, B_LOC], F32, tag="pt", name="pt")
            nc.tensor.transpose(pt[:], encs[:, k * 128:(k + 1) * 128],
                                ident[0:B_LOC, 0:B_LOC])
            nc.vector.tensor_copy(h0T[:, k * HS: k * HS + B_LOC], pt[:])
        for k in range(KC):
            nc.vector.memset(h1T[:, k * HS: k * HS + B_LOC], 0.0)

        # ---------- x load + transpose to xT[k] = [128, TB] bf16 ----------
        xT = [sb.tile([128, TB], BF16, tag=f"xT{k}", name=f"xT{k}")
              for k in range(KC)]

        xg_box = {}

        def load_x_group(g):
            """DMA 4 j-blocks (512 TB-cols) of x into one staging buffer."""
            xs = xstg.tile([128, 2048], F32, tag="xg", name="xg")
            for j in range(4):
                t0 = g * 64 + j * 16
                nc.sync.dma_start(
                    xs[:, j * 512:(j + 1) * 512],
                    x_d[:, t0:t0 + 16, :].rearrange("b t d -> t b d"))
            xg_box[g] = xs

        def transpose_x_chunk(g, j):
            """Transpose 128-col chunk j (0..3) of group g into xT."""
            xs = xg_box[g]
            jj = g * 4 + j
            for k in range(KC):
                pt = ps_t.tile([128, 128], F32, tag="pt", name="pt")
                nc.tensor.transpose(
                    pt[:], xs[:, j * 512 + k * 128: j * 512 + (k + 1) * 128],
                    ident[:])
                nc.vector.tensor_copy(
                    xT[k][:, jj * 128:(jj + 1) * 128], pt[:])

        # DMA order on the sync queue: x-g0 first (gates the transposes),
        # then wx0 (gates P0 n=0), then whh0 (gates rec0 step 0)
        load_x_group(0)
        wx0 = load_half(wh0_d, 0, "wx0")
        whh0 = load_half(wh0_d, D, "whh0")
        for j in range(4):
            transpose_x_chunk(0, j)

        # ---------- P stores (single bf16): P = X @ Whx + bh ----------
        P0 = sb.tile([128, T * 32], BF16, tag="P0", name="P0")
        P1 = sb.tile([128, T * 32], BF16, tag="P1", name="P1")

        def p_view(P):
            return P[:].rearrange("p (t m b) -> p t m b", m=KC, b=B_LOC)

        def emit_p_block(P, w, src, bias, m, n, wcols):
            """One (m, n) block of a P GEMM; returns list of emit thunks."""
            thunks = []
            pg_box = []
            ts = wcols // B_LOC

            def mk_mm(k):
                def f(dep=None):
                    if k == 0:
                        pg_box.append(ps_g.tile([128, wcols], F32, tag="pg",
                                                name="pg"))
                    mm = nc.tensor.matmul(
                        pg_box[0][:],
                        w[:, k * 512 + m * 128: k * 512 + (m + 1) * 128],
                        src[k][:, n * wcols:(n + 1) * wcols],
                        start=(k == 0), stop=(k == KC - 1),
                        skip_group_check=True)
                    if dep is not None:
                        add_dep_helper(mm.ins, dep, sync=False, reason="spread")
                return f

            for k in range(KC):
                thunks.append(mk_mm(k))

            def epi(dep=None):
                pg = pg_box[0]
                nc.vector.tensor_scalar_add(
                    p_view(P)[:, n * ts:(n + 1) * ts, m, :],
                    pg[:].rearrange("p (t b) -> p t b", b=B_LOC),
                    bias[:, m: m + 1])
            thunks.append(epi)
            return thunks

        # ---------- o GEMM: tanh(X@Wox + Hprev@Woh + bo) -> bf16 [feat, TB] ----------
        def emit_o_block(dst, wx, wh, hT, bias, m, n, wcols):
            thunks = []
            pg_box = []

            def mk_x(k):
                def f(dep=None):
                    if k == 0:
                        pg_box.append(ps_g.tile([128, wcols], F32, tag="pg",
                                                name="pg"))
                    mm = nc.tensor.matmul(
                        pg_box[0][:],
                        wx[:, k * 512 + m * 128: k * 512 + (m + 1) * 128],
                        xT[k][:, n * wcols:(n + 1) * wcols],
                        start=(k == 0), stop=False, skip_group_check=True)
                    if dep is not None:
                        add_dep_helper(mm.ins, dep, sync=False, reason="spread")
                return f

            def mk_h(k):
                def f(dep=None):
                    mm = nc.tensor.matmul(
                        pg_box[0][:],
                        wh[:, k * 512 + m * 128: k * 512 + (m + 1) * 128],
                        hT[:, k * HS + n * wcols: k * HS + (n + 1) * wcols],
                        start=False, stop=(k == KC - 1), skip_group_check=True)
                    if dep is not None:
                        add_dep_helper(mm.ins, dep, sync=False, reason="spread")
                return f

            for k in range(KC):
                thunks.append(mk_x(k))
            for k in range(KC):
                thunks.append(mk_h(k))

            def epi(dep=None):
                nc.scalar.activation(dst[m][:, n * wcols:(n + 1) * wcols],
                                     pg_box[0][:], Tanh, bias=bias[:, m: m + 1])
            thunks.append(epi)
            return thunks

        out0T = [sb.tile([128, TB], BF16, tag=f"o0T{m}", name=f"o0T{m}")
                 for m in range(KC)]

        # ---------- final output block ([TB, feat] row-major) ----------
        def emit_out_block(mt):
            thunks = []
            po_box = []

            def bias_mm(dep=None):
                po_box.append(ps_g.tile([128, 512], F32, tag="pg", name="pg"))
                mm = nc.tensor.matmul(po_box[0][:], ones_b[:], bo1b[:],
                                 start=True, stop=False, skip_group_check=True)
                if dep is not None:
                    add_dep_helper(mm.ins, dep, sync=False, reason="spread")
            thunks.append(bias_mm)

            def mk_x(k):
                def f(dep=None):
                    mm = nc.tensor.matmul(
                        po_box[0][:], out0T[k][:, mt * 128:(mt + 1) * 128],
                        wo1x[:, k * 512:(k + 1) * 512],
                        start=False, stop=False, skip_group_check=True)
                    if dep is not None:
                        add_dep_helper(mm.ins, dep, sync=False, reason="spread")
                return f

            def mk_h(k):
                def f(dep=None):
                    mm = nc.tensor.matmul(
                        po_box[0][:],
                        h1T[:, k * HS + mt * 128: k * HS + (mt + 1) * 128],
                        woh1[:, k * 512:(k + 1) * 512],
                        start=False, stop=(k == KC - 1), skip_group_check=True)
                    if dep is not None:
                        add_dep_helper(mm.ins, dep, sync=False, reason="spread")
                return f

            for k in range(KC):
                thunks.append(mk_x(k))
            for k in range(KC):
                thunks.append(mk_h(k))

            def epi(dep=None):
                orow = stg.tile([128, 512], F32, tag="orow", name="orow")
                nc.scalar.activation(orow[:], po_box[0][:], Tanh)
                nc.sync.dma_start(
                    out_d[:, mt * 16:(mt + 1) * 16, :].rearrange("b t d -> t b d"),
                    orow[:])
            thunks.append(epi)
            return thunks

        # ---------- P0 n=0 up-front (needed from step 0) ----------
        for m in range(KC):
            for fn in emit_p_block(P0, wx0, xT, bh0, m, 0, 512):
                fn()

        # ---------- remaining DMAs (queue behind the prologue x/weights) ----------
        wox0 = load_half(wo0_d, 0, "wox0")
        woh0 = load_half(wo0_d, D, "woh0")
        if NG > 1:
            load_x_group(1)
        wx1 = load_half(wh1_d, 0, "wx1")
        whh1 = load_half(wh1_d, D, "whh1")
        wo1x = load_half(wo1_d, 0, "wo1x")
        woh1 = load_half(wo1_d, D, "woh1")

        # ---------- recurrence step ----------
        # z-bank warmup: one start=True matmul per z buffer slot sets the
        # bank's has_written bits. After that, every step's DVE prewrite
        # overwrites the memory with P_t (bits stay set), and the whh
        # matmuls (all flags=0) accumulate on top — no identity matmuls on
        # the PE critical path.
        for zpool in (ps_z0, ps_z1):
            for _ in range(2):
                zw = zpool.tile([128, 32], F32, tag="z", name="z")
                nc.tensor.matmul(
                    zw[:], ident_b[:], ident_b[:, 0:32],
                    start=True, stop=True, skip_group_check=True)

        def rec_step(zpool, P, hTa, whh, t):
            hview = hTa[:].rearrange("p (c s) -> p c s", c=KC)
            z = zpool.tile([128, 32], F32, tag="z", name="z")
            nc.vector.tensor_copy(
                z[:].rearrange("p (m b) -> p m b", b=B_LOC),
                p_view(P)[:, t, :, :])
            for k in range(KC):
                for m in range(KC):
                    nc.tensor.matmul(
                        z[:, m * 8:(m + 1) * 8],
                        whh[:, k * 512 + m * 128: k * 512 + (m + 1) * 128],
                        hTa[:, k * HS + t * 8: k * HS + (t + 1) * 8],
                        start=False, stop=(k == KC - 1 and m == KC - 1),
                        skip_group_check=True)
            act = nc.scalar.activation(
                hview[:, :, (t + 1) * 8:(t + 2) * 8],
                z[:].rearrange("p (c b) -> p c b", b=B_LOC),
                Tanh)
            return act

        # ---------- fill schedule ----------
        # Emission ORDER is what forms Tile dependencies: every fill thunk
        # that WRITES data a recurrence step reads (P epilogues, x
        # transposes) must be EMITTED before that step's instructions.
        # Items carry (earliest, deadline): `earliest` = first superstep at
        # which emission is dependency-correct (all writers this item reads
        # are already emitted); `deadline` = superstep of the first consumer
        # (mandatory drain happens just before it). FIFO is deadline-sorted;
        # opportunistic emission only from the head item (never skips).
        INF = 10 ** 9
        items = []   # (deadline, seq, earliest, [thunks])

        def add_item(earliest, deadline, thunks):
            items.append([deadline, len(items), earliest, list(thunks)])

        # x groups g>=1: DMA then transposes (consumed by P0 block g and
        # o0 blocks 2g/2g+1; all deadline-protected via P0's deadline 64g)
        for g in range(1, NG):
            def mk_dma(gg):
                def f(dep=None):
                    load_x_group(gg)
                return f
            def mk_tr(gg, jj):
                def f(dep=None):
                    transpose_x_chunk(gg, jj)
                return f
            thl = [] if g == 1 else [mk_dma(g)]
            thl += [mk_tr(g, j) for j in range(4)]
            add_item(20 * (g - 1) + 10, 64 * g - 8, thl)

        # deferred P0 blocks (cols n*64.. needed from rec0 step n*64)
        for n in range(1, NB):
            thl = []
            for m in range(KC):
                thl += emit_p_block(P0, wx0, xT, bh0, m, n, 512)
            add_item(20 * (n - 1) + 26, 64 * n, thl)

        # o0 + P1 blocks (n2-th reads h0 through step (n2+1)*32 - 1, so
        # earliest emission is ss 32*(n2+1); P1 block n2 is read by rec1
        # step 32*n2 at ss LAG + 32*n2)
        for n2 in range(NB2):
            thl = []
            for m in range(KC):
                thl += emit_o_block(out0T, wox0, woh0, h0T, bo0, m, n2, 256)
            for m in range(KC):
                thl += emit_p_block(P1, wx1, out0T, bh1, m, n2, 256)
            add_item(32 * (n2 + 1), LAG + 32 * n2, thl)

        # final out blocks (read-only consumers of h1T/out0T; mt-th reads
        # h1 through step (mt+1)*16 - 1, emitted at ss LAG+16*(mt+1)-1)
        for mt in range(MT):
            add_item(LAG + 16 * (mt + 1), INF, emit_out_block(mt))

        items.sort(key=lambda it: (it[0], it[1]))

        # ---------- merged superstep loop ----------
        # Opportunistic emission pulls from the FIFO head; additionally ONE
        # read-only OUT item (deadline INF, writers all emitted before its
        # earliest) may run as a skip-ahead stream when the head is not yet
        # eligible. At most two fill blocks are ever open at once, matching
        # ps_g's two buffers.
        CAP = 3
        skip_it = None
        for s in range(S_TOT):
            pend = []
            # mandatory: drain everything whose consumer runs this superstep
            while items and items[0][0] <= s:
                pend += items.pop(0)[3]
            act0 = act1 = None
            if s < T:
                act0 = rec_step(ps_z0, P0, h0T, whh0, s)
            cap = CAP if s < T else 6
            budget = max(0, cap - len(pend))
            while budget > 0:
                if items and items[0][2] <= s:
                    head = items[0][3]
                    pend.append(head.pop(0))
                    budget -= 1
                    if not head:
                        items.pop(0)
                    continue
                if skip_it is None:
                    for it in items:
                        if it[0] >= INF and it[2] <= s:
                            skip_it = it
                            break
                if skip_it is not None:
                    pend.append(skip_it[3].pop(0))
                    budget -= 1
                    if not skip_it[3]:
                        items.remove(skip_it)
                        skip_it = None
                    continue
                break
            dep0 = act0.ins if act0 is not None else None
            for th in pend:
                th(dep0)
            if LAG <= s < T + LAG:
                act1 = rec_step(ps_z1, P1, h1T, whh1, s - LAG)
            if act0 is None and pend:
                # fills on tail-only supersteps hang off act1 instead
                pass

        for it in items:
            for th in it[3]:
                th()

    nc.compile()
    return nc


_NC_CACHE = {}


def _get_nc(T=256):
    if T not in _NC_CACHE:
        _NC_CACHE[T] = build_kernel(T)
    return _NC_CACHE[T]


def kernel(**inputs):
    x = np.ascontiguousarray(inputs["x"], dtype=np.float32)
    enc = np.ascontiguousarray(inputs["encoder_output"], dtype=np.float32)
    B, T, _ = x.shape
    nc = _get_nc(T)
    shared = {
        "Wh0": np.ascontiguousarray(inputs["Wh0"], np.float32),
        "bh0": np.ascontiguousarray(inputs["bh0"], np.float32),
        "Wo0": np.ascontiguousarray(inputs["Wo0"], np.float32),
        "bo0": np.ascontiguousarray(inputs["bo0"], np.float32),
        "Wh1": np.ascontiguousarray(inputs["Wh1"], np.float32),
        "bh1": np.ascontiguousarray(inputs["bh1"], np.float32),
        "Wo1": np.ascontiguousarray(inputs["Wo1"], np.float32),
        "bo1": np.ascontiguousarray(inputs["bo1"], np.float32),
    }
    in_maps = []
    for c in range(N_CORES):
        in_maps.append({
            "x": x[c * B_LOC:(c + 1) * B_LOC],
            "encoder_output": enc[c * B_LOC:(c + 1) * B_LOC],
            **shared,
        })
    res = run_bass_kernel_spmd(nc, in_maps, core_ids=list(range(N_CORES)))
    out = np.concatenate([res.results[c]["out"] for c in range(N_CORES)], axis=0)
    return out.astype(np.float32)
